# revision 2
# baseline (speedup 1.0000x reference)
"""Multi-head causal attention with RoPE on 8 TRN2 NeuronCores.

Sharding: 8 cores = 2 batches x 4 head-groups (4 heads each).
Per-core Bass kernel computes the group-partial output transposed;
host sums group partials and transposes back.

All matmuls run in float32r (fp32 storage, ~2e-4 relative error,
~bf16 throughput at moving-dim 512).
"""

import numpy as np

import concourse.bass as bass  # noqa: F401  (AP helpers)
import concourse.tile as tile
from concourse import bacc, mybir

# Problem shape (hardcoded per contest rules)
B, S, D, H, HD = 2, 2048, 2048, 16, 128
NCORES = 8
G = 4            # head groups
GH = 4           # heads per group
GD = GH * HD     # 512 dims per group
P = 128          # partitions
SQ_U = S // 512  # 4 query slices
SK_T = S // P    # 16 key tiles

_f32 = mybir.dt.float32
_f32r = mybir.dt.float32r

_cache = {}  # causal(bool) -> BassRunner


def _build(causal: bool):
    nc = bacc.Bacc("TRN2", target_bir_lowering=False, debug=False)
    xT = nc.dram_tensor("xT", [D, S], _f32r, kind="ExternalInput").ap()
    wq = nc.dram_tensor("wq", [D, GD], _f32r, kind="ExternalInput").ap()
    wk = nc.dram_tensor("wk", [D, GD], _f32r, kind="ExternalInput").ap()
    wv = nc.dram_tensor("wv", [D, GD], _f32r, kind="ExternalInput").ap()
    wo = nc.dram_tensor("wo", [GD, D], _f32r, kind="ExternalInput").ap()
    cs = nc.dram_tensor("cs", [P, S], _f32, kind="ExternalInput").ap()
    ss = nc.dram_tensor("ss", [P, S], _f32, kind="ExternalInput").ap()
    ones = nc.dram_tensor("ones", [P, P], _f32r, kind="ExternalInput").ap()
    if causal:
        maskd = nc.dram_tensor("maskd", [P, SK_T * 512], _f32, kind="ExternalInput").ap()
    else:
        maskf = nc.dram_tensor("maskf", [S, S], _f32, kind="ExternalInput").ap()
    outT = nc.dram_tensor("outT", [D, S], _f32, kind="ExternalOutput").ap()

    with tile.TileContext(nc) as tc:
        with (
            tc.tile_pool(name="persist", bufs=1) as persist,
            tc.tile_pool(name="dram", bufs=1, space="DRAM") as dpool,
        ):
            qtd = dpool.tile([P, GH * S], _f32r, tag="qtd")   # Q^T per head [hd, S]
            ktd = dpool.tile([P, GH * S], _f32r, tag="ktd")
            vd = dpool.tile([P, SK_T * GD], _f32r, tag="vd")  # V natural [s-tile, 4*hd]
            ones_s = persist.tile([P, P], _f32r, tag="ones")
            nc.sync.dma_start(ones_s[:], ones[:])
            aot = persist.tile([P, GH * S], _f32r, tag="aot")  # AO^T per head

            # ---- Phase P1: Q^T / K^T projections + RoPE -> DRAM scratch
            with (
                tc.tile_pool(name="p1w", bufs=1) as p1w,
                tc.tile_pool(name="p1x", bufs=2) as p1x,
                tc.tile_pool(name="p1t", bufs=3) as p1t,
                tc.tile_pool(name="p1o", bufs=4) as p1o,
                tc.tile_pool(name="ps1", bufs=4, space="PSUM") as ps1,
            ):
                wq_s = p1w.tile([P, SK_T * GD], _f32r, tag="wq")
                wk_s = p1w.tile([P, SK_T * GD], _f32r, tag="wk")
                cs_s = p1w.tile([P, S], _f32, tag="cs")
                ss_s = p1w.tile([P, S], _f32, tag="ss")
                nc.sync.dma_start(cs_s[:], cs[:])
                nc.sync.dma_start(ss_s[:], ss[:])
                for t in range(SK_T):
                    nc.sync.dma_start(wq_s[:, t * GD:(t + 1) * GD], wq[t * P:(t + 1) * P, :])
                    nc.sync.dma_start(wk_s[:, t * GD:(t + 1) * GD], wk[t * P:(t + 1) * P, :])
                for u in range(SQ_U):
                    # stream x^T slice in two halves (SBUF budget)
                    xh = []
                    for half in range(2):
                        xu = p1x.tile([P, 8 * 512], _f32r, tag="xu")
                        for j in range(8):
                            t = half * 8 + j
                            nc.sync.dma_start(
                                xu[:, j * 512:(j + 1) * 512],
                                xT[t * P:(t + 1) * P, u * 512:(u + 1) * 512])
                        xh.append(xu)
                    su = slice(u * 512, (u + 1) * 512)
                    for (w_s, dst) in ((wq_s, qtd), (wk_s, ktd)):
                        for dt in range(GH):
                            pq = ps1.tile([P, 512], _f32, tag="pq")
                            for t in range(SK_T):
                                xu = xh[t // 8]
                                j = t % 8
                                nc.tensor.matmul(
                                    pq[:],
                                    w_s[:, t * GD + dt * P: t * GD + dt * P + P],
                                    xu[:, j * 512:(j + 1) * 512],
                                    start=(t == 0), stop=(t == SK_T - 1))
                            # RoPE: rot = pq*cs + swap(pq)*ss  (ss signed)
                            t1 = p1t.tile([P, 512], _f32, tag="t1")
                            t2 = p1t.tile([P, 512], _f32, tag="t2")
                            nc.vector.tensor_mul(t1[:], pq[:], cs_s[:, su])
                            nc.vector.tensor_mul(t2[0:64, :], pq[64:P, :], ss_s[0:64, su])
                            nc.vector.tensor_mul(t2[64:P, :], pq[0:64, :], ss_s[64:P, su])
                            ro = p1o.tile([P, 512], _f32r, tag="ro")
                            nc.vector.tensor_add(ro[:], t1[:], t2[:])
                            nc.sync.dma_start(
                                dst[:, dt * S + u * 512: dt * S + (u + 1) * 512], ro[:])

            # ---- Phase P2: V projection (natural layout) -> DRAM scratch
            with (
                tc.tile_pool(name="p2w", bufs=1) as p2w,
                tc.tile_pool(name="p2x", bufs=2) as p2x,
                tc.tile_pool(name="p2o", bufs=4) as p2o,
                tc.tile_pool(name="ps2", bufs=4, space="PSUM") as ps2,
            ):
                wv_s = p2w.tile([P, SK_T * GD], _f32r, tag="wv")
                for t in range(SK_T):
                    nc.sync.dma_start(wv_s[:, t * GD:(t + 1) * GD], wv[t * P:(t + 1) * P, :])
                for u in range(SQ_U):
                    xh = []
                    for half in range(2):
                        xu = p2x.tile([P, 8 * 512], _f32r, tag="xu")
                        for j in range(8):
                            t = half * 8 + j
                            nc.sync.dma_start(
                                xu[:, j * 512:(j + 1) * 512],
                                xT[t * P:(t + 1) * P, u * 512:(u + 1) * 512])
                        xh.append(xu)
                    for st in range(4):
                        g = 4 * u + st
                        pv = ps2.tile([P, GD], _f32, tag="pv")
                        for t in range(SK_T):
                            xu = xh[t // 8]
                            j = t % 8
                            nc.tensor.matmul(
                                pv[:],
                                xu[:, j * 512 + st * P: j * 512 + st * P + P],
                                wv_s[:, t * GD:(t + 1) * GD],
                                start=(t == 0), stop=(t == SK_T - 1))
                        vo = p2o.tile([P, GD], _f32r, tag="vo")
                        nc.scalar.copy(vo[:], pv[:])
                        nc.sync.dma_start(vd[:, g * GD:(g + 1) * GD], vo[:])

            # ---- Phases A (attention) + W (output projection)
            with (
                tc.tile_pool(name="amask", bufs=1 if causal else 2) as amask_p,
                tc.tile_pool(name="akv", bufs=2) as akv,
                tc.tile_pool(name="aq", bufs=2) as aq,
                tc.tile_pool(name="app", bufs=4) as app,
                tc.tile_pool(name="ar", bufs=2) as ar,
                tc.tile_pool(name="aw", bufs=1) as aw,
                tc.tile_pool(name="wst", bufs=3) as wst,
                tc.tile_pool(name="ps3", bufs=2, space="PSUM") as ps3,
                tc.tile_pool(name="ps4", bufs=2, space="PSUM") as ps4,
            ):
                wo_s = aw.tile([P, GH * S], _f32r, tag="wo")
                for dt in range(GH):
                    nc.sync.dma_start(wo_s[:, dt * S:(dt + 1) * S], wo[dt * P:(dt + 1) * P, :])
                if causal:
                    md_s = amask_p.tile([P, SK_T * 512], _f32, tag="md")
                    nc.sync.dma_start(md_s[:], maskd[:])
                for h in range(GH):
                    kt_h = akv.tile([P, S], _f32r, tag="kt")
                    nc.sync.dma_start(kt_h[:], ktd[:, h * S:(h + 1) * S])
                    v_h = akv.tile([P, S], _f32r, tag="vh")
                    for t in range(SK_T):
                        nc.sync.dma_start(
                            v_h[:, t * P:(t + 1) * P],
                            vd[:, t * GD + h * P: t * GD + (h + 1) * P])
                    for u in range(SQ_U):
                        qu = aq.tile([P, 512], _f32r, tag="qu")
                        nc.sync.dma_start(
                            qu[:], qtd[:, h * S + u * 512: h * S + (u + 1) * 512])
                        if not causal:
                            mu = amask_p.tile([P, SK_T * 512], _f32, tag="mu")
                            for t in range(SK_T):
                                nc.sync.dma_start(
                                    mu[:, t * 512:(t + 1) * 512],
                                    maskf[t * P:(t + 1) * P, u * 512:(u + 1) * 512])
                        n_sk = 4 * (u + 1) if causal else SK_T
                        psd = ps3.tile([P, 512], _f32, tag="d")
                        psa = ps3.tile([P, 512], _f32, tag="a")
                        pts = [None] * n_sk

                        def _consume(t):
                            nc.tensor.matmul(psd[:], ones_s[:], pts[t][:],
                                             start=(t == 0), stop=(t == n_sk - 1))
                            nc.tensor.matmul(psa[:], v_h[:, t * P:(t + 1) * P], pts[t][:],
                                             start=(t == 0), stop=(t == n_sk - 1))

                        for t in range(n_sk):
                            pss = ps3.tile([P, 512], _f32, tag="s")
                            nc.tensor.matmul(pss[:], kt_h[:, t * P:(t + 1) * P], qu[:],
                                             start=True, stop=True)
                            if causal:
                                if t >= 4 * u:  # diagonal-crossing blocks
                                    nc.vector.tensor_add(
                                        pss[:], pss[:], md_s[:, t * 512:(t + 1) * 512])
                            else:
                                nc.vector.tensor_add(
                                    pss[:], pss[:], mu[:, t * 512:(t + 1) * 512])
                            pt = app.tile([P, 512], _f32r, tag="p")
                            nc.scalar.activation(pt[:], pss[:],
                                                 mybir.ActivationFunctionType.Exp)
                            pts[t] = pt
                            if t > 0:
                                _consume(t - 1)  # software-pipeline: hide exp latency
                        _consume(n_sk - 1)
                        rec = ar.tile([P, 512], _f32, tag="rec")
                        nc.vector.reciprocal(rec[:], psd[:])
                        nc.vector.tensor_mul(
                            aot[:, h * S + u * 512: h * S + (u + 1) * 512],
                            psa[:], rec[:])
                # ---- W: out^T = wo^T.T @ AO^T
                for u in range(SQ_U):
                    for ot in range(SK_T):
                        po = ps4.tile([P, 512], _f32, tag="o")
                        for dt in range(GH):
                            nc.tensor.matmul(
                                po[:],
                                wo_s[:, dt * S + ot * P: dt * S + (ot + 1) * P],
                                aot[:, dt * S + u * 512: dt * S + (u + 1) * 512],
                                start=(dt == 0), stop=(dt == GH - 1))
                        so = wst.tile([P, 512], _f32, tag="so")
                        nc.scalar.copy(so[:], po[:])
                        nc.sync.dma_start(
                            outT[ot * P:(ot + 1) * P, u * 512:(u + 1) * 512], so[:])
    nc.compile()
    return nc


class _Runner:
    """Persistent PJRT executable for one compiled Bass module (SPMD over 8 cores)."""

    def __init__(self, nc, n_cores):
        import jax
        from jax.sharding import Mesh, PartitionSpec
        from jax.experimental.shard_map import shard_map
        from concourse.bass2jax import (
            _bass_exec_p, install_neuronx_cc_hook, partition_id_tensor)

        install_neuronx_cc_hook()
        self.jax = jax
        self.n_cores = n_cores
        partition_name = nc.partition_id_tensor.name if nc.partition_id_tensor else None
        in_names, out_names, out_avals = [], [], []
        for alloc in nc.m.functions[0].allocations:
            if not isinstance(alloc, mybir.MemoryLocationSet):
                continue
            name = alloc.memorylocations[0].name
            if alloc.kind == "ExternalInput":
                if name != partition_name:
                    in_names.append(name)
            elif alloc.kind == "ExternalOutput":
                out_names.append(name)
                out_avals.append(jax.core.ShapedArray(
                    tuple(alloc.tensor_shape), mybir.dt.np(alloc.dtype)))
        self.in_names, self.out_names, self.out_avals = in_names, out_names, out_avals
        n_params, n_outs = len(in_names), len(out_avals)
        all_in = list(in_names) + list(out_names)
        if partition_name is not None:
            all_in.append(partition_name)

        def _body(*args):
            operands = list(args)
            if partition_name is not None:
                operands.append(partition_id_tensor())
            return tuple(_bass_exec_p.bind(
                *operands,
                out_avals=tuple(out_avals), in_names=tuple(all_in),
                out_names=tuple(out_names), lowering_input_output_aliases=(),
                sim_require_finite=True, sim_require_nnan=True, nc=nc))

        devices = jax.devices()[:n_cores]
        mesh = Mesh(np.asarray(devices), ("core",))
        self.fn = jax.jit(
            shard_map(_body, mesh=mesh,
                      in_specs=(PartitionSpec("core"),) * (n_params + n_outs),
                      out_specs=(PartitionSpec("core"),) * n_outs,
                      check_rep=False),
            keep_unused=True)
        self._dev_args = None

    def put_inputs(self, in_maps):
        jax = self.jax
        concat_in = [
            np.concatenate([np.asarray(in_maps[c][n]) for c in range(self.n_cores)], axis=0)
            for n in self.in_names]
        concat_zeros = [
            np.zeros((self.n_cores * a.shape[0], *a.shape[1:]), a.dtype)
            for a in self.out_avals]
        self._dev_args = [jax.device_put(v) for v in concat_in + concat_zeros]
        for a in self._dev_args:
            a.block_until_ready()

    def execute(self):
        return self.fn(*self._dev_args)

    def run(self, in_maps):
        self.put_inputs(in_maps)
        outs = self.execute()
        self.jax.block_until_ready(outs)
        return [
            {n: np.asarray(outs[i]).reshape(self.n_cores, *self.out_avals[i].shape)[c]
             for i, n in enumerate(self.out_names)}
            for c in range(self.n_cores)]


def _get_runner(causal: bool):
    if causal not in _cache:
        _cache[causal] = _Runner(_build(causal), NCORES)
    return _cache[causal]


def _host_prep(x, mask, Wq, Wk, Wv, Wo, causal):
    scale = np.float32(1.0) / np.sqrt(np.float32(HD))
    # de-interleave permutation within each head (cancels in Q.K^T)
    perm = np.concatenate(
        [np.concatenate([np.arange(0, HD, 2), np.arange(1, HD, 2)]) + HD * hh
         for hh in range(GH)])
    # RoPE tables, duplicated across halves; sin signed for fused add
    inv = (np.float32(1.0) / np.power(
        np.float32(10000.0),
        np.arange(0, HD, 2).astype(np.float32) / np.float32(HD))).astype(np.float32)
    ang = np.arange(S, dtype=np.float32)[:, None] * inv[None, :]   # [S, 64]
    cos_t = np.cos(ang).T.astype(np.float32)                       # [64, S]
    sin_t = np.sin(ang).T.astype(np.float32)
    cs_host = np.ascontiguousarray(np.concatenate([cos_t, cos_t], axis=0))
    ss_host = np.ascontiguousarray(np.concatenate([-sin_t, sin_t], axis=0))
    ones_host = np.ones((P, P), np.float32)
    maskT = np.ascontiguousarray(mask.T)
    if causal:
        md = np.empty((P, SK_T * 512), np.float32)
        for t in range(SK_T):
            u = t // 4
            md[:, t * 512:(t + 1) * 512] = maskT[t * P:(t + 1) * P, u * 512:(u + 1) * 512]
    xTs = [np.ascontiguousarray(x[b].T) for b in range(B)]
    in_maps = []
    for c in range(NCORES):
        b, g = c // G, c % G
        rows = slice(g * GD, (g + 1) * GD)
        m = {
            "xT": xTs[b],
            "wq": np.ascontiguousarray(Wq[rows].T[:, perm] * scale),
            "wk": np.ascontiguousarray(Wk[rows].T[:, perm]),
            "wv": np.ascontiguousarray(Wv[rows].T),
            "wo": np.ascontiguousarray(Wo[:, rows].T),
            "cs": cs_host,
            "ss": ss_host,
            "ones": ones_host,
        }
        if causal:
            m["maskd"] = md
        else:
            m["maskf"] = maskT
        in_maps.append(m)
    return in_maps


def kernel(x, mask, Wq, Wk, Wv, Wo):
    x = np.asarray(x, dtype=np.float32)
    mask = np.asarray(mask, dtype=np.float32)
    Wq = np.asarray(Wq, dtype=np.float32)
    Wk = np.asarray(Wk, dtype=np.float32)
    Wv = np.asarray(Wv, dtype=np.float32)
    Wo = np.asarray(Wo, dtype=np.float32)
    expected_mask = np.triu(np.full((S, S), -1e9, dtype=np.float32), k=1)
    causal = bool(np.array_equal(mask, expected_mask))
    runner = _get_runner(causal)
    in_maps = _host_prep(x, mask, Wq, Wk, Wv, Wo, causal)
    results = runner.run(in_maps)
    out = np.empty((B, S, D), np.float32)
    for b in range(B):
        acc = results[b * G]["outT"].copy()
        for g in range(1, G):
            acc += results[b * G + g]["outT"]
        out[b] = acc.T
    return out


# revision 3
# speedup vs baseline: 1.5128x; 1.5128x over previous
"""Multi-head causal attention with RoPE on 8 TRN2 NeuronCores.

Sharding: 8 cores = 2 batches x 4 head-groups (4 heads each).
Per-core Bass kernel computes the group-partial output transposed;
host sums group partials and transposes back.

All matmuls run in float32r (fp32 storage, ~2e-4 relative error,
~bf16 throughput at moving-dim 512).
"""

import numpy as np

import concourse.bass as bass  # noqa: F401  (AP helpers)
import concourse.tile as tile
from concourse import bacc, mybir

# Problem shape (hardcoded per contest rules)
B, S, D, H, HD = 2, 2048, 2048, 16, 128
NCORES = 8
G = 4            # head groups
GH = 4           # heads per group
GD = GH * HD     # 512 dims per group
P = 128          # partitions
SQ_U = S // 512  # 4 query slices
SK_T = S // P    # 16 key tiles

_f32 = mybir.dt.float32
_f32r = mybir.dt.float32r

_cache = {}  # causal(bool) -> BassRunner


def _build(causal: bool):
    nc = bacc.Bacc("TRN2", target_bir_lowering=False, debug=False)
    xT = nc.dram_tensor("xT", [D, S], _f32r, kind="ExternalInput").ap()
    wq = nc.dram_tensor("wq", [D, GD], _f32r, kind="ExternalInput").ap()
    wk = nc.dram_tensor("wk", [D, GD], _f32r, kind="ExternalInput").ap()
    wv = nc.dram_tensor("wv", [D, GD], _f32r, kind="ExternalInput").ap()
    wo = nc.dram_tensor("wo", [GD, D], _f32r, kind="ExternalInput").ap()
    cs = nc.dram_tensor("cs", [P, S], _f32, kind="ExternalInput").ap()
    ss = nc.dram_tensor("ss", [P, S], _f32, kind="ExternalInput").ap()
    ones = nc.dram_tensor("ones", [P, P], _f32r, kind="ExternalInput").ap()
    if causal:
        maskd = nc.dram_tensor("maskd", [P, SK_T * 512], _f32, kind="ExternalInput").ap()
    else:
        maskf = nc.dram_tensor("maskf", [S, S], _f32, kind="ExternalInput").ap()
    outT = nc.dram_tensor("outT", [D, S], _f32, kind="ExternalOutput").ap()

    with tile.TileContext(nc) as tc:
        with (
            tc.tile_pool(name="persist", bufs=1) as persist,
            tc.tile_pool(name="dram", bufs=1, space="DRAM") as dpool,
        ):
            qtd = dpool.tile([P, GH * S], _f32r, tag="qtd")   # Q^T per head [hd, S]
            ktd = dpool.tile([P, GH * S], _f32r, tag="ktd")
            vd = dpool.tile([P, SK_T * GD], _f32r, tag="vd")  # V natural [s-tile, 4*hd]
            ones_s = persist.tile([P, P], _f32r, tag="ones")
            nc.sync.dma_start(ones_s[:], ones[:])
            aot = persist.tile([P, GH * S], _f32r, tag="aot")  # AO^T per head

            # ---- Phase P1: Q^T / K^T projections + RoPE -> DRAM scratch
            with (
                tc.tile_pool(name="p1w", bufs=1) as p1w,
                tc.tile_pool(name="p1x", bufs=2) as p1x,
                tc.tile_pool(name="p1t", bufs=3) as p1t,
                tc.tile_pool(name="p1o", bufs=4) as p1o,
                tc.tile_pool(name="ps1", bufs=4, space="PSUM") as ps1,
            ):
                wq_s = p1w.tile([P, SK_T * GD], _f32r, tag="wq")
                wk_s = p1w.tile([P, SK_T * GD], _f32r, tag="wk")
                cs_s = p1w.tile([P, S], _f32, tag="cs")
                ss_s = p1w.tile([P, S], _f32, tag="ss")
                nc.sync.dma_start(cs_s[:], cs[:])
                nc.sync.dma_start(ss_s[:], ss[:])
                for t in range(SK_T):
                    nc.sync.dma_start(wq_s[:, t * GD:(t + 1) * GD], wq[t * P:(t + 1) * P, :])
                    nc.sync.dma_start(wk_s[:, t * GD:(t + 1) * GD], wk[t * P:(t + 1) * P, :])
                for u in range(SQ_U):
                    # stream x^T slice in two halves (SBUF budget)
                    xh = []
                    for half in range(2):
                        xu = p1x.tile([P, 8 * 512], _f32r, tag="xu")
                        for j in range(8):
                            t = half * 8 + j
                            nc.sync.dma_start(
                                xu[:, j * 512:(j + 1) * 512],
                                xT[t * P:(t + 1) * P, u * 512:(u + 1) * 512])
                        xh.append(xu)
                    su = slice(u * 512, (u + 1) * 512)
                    for (w_s, dst) in ((wq_s, qtd), (wk_s, ktd)):
                        for dt in range(GH):
                            pq = ps1.tile([P, 512], _f32, tag="pq")
                            for t in range(SK_T):
                                xu = xh[t // 8]
                                j = t % 8
                                nc.tensor.matmul(
                                    pq[:],
                                    w_s[:, t * GD + dt * P: t * GD + dt * P + P],
                                    xu[:, j * 512:(j + 1) * 512],
                                    start=(t == 0), stop=(t == SK_T - 1))
                            # RoPE: rot = pq*cs + swap(pq)*ss  (ss signed)
                            t1 = p1t.tile([P, 512], _f32, tag="t1")
                            t2 = p1t.tile([P, 512], _f32, tag="t2")
                            nc.vector.tensor_mul(t1[:], pq[:], cs_s[:, su])
                            nc.vector.tensor_mul(t2[0:64, :], pq[64:P, :], ss_s[0:64, su])
                            nc.vector.tensor_mul(t2[64:P, :], pq[0:64, :], ss_s[64:P, su])
                            ro = p1o.tile([P, 512], _f32r, tag="ro")
                            nc.vector.tensor_add(ro[:], t1[:], t2[:])
                            nc.sync.dma_start(
                                dst[:, dt * S + u * 512: dt * S + (u + 1) * 512], ro[:])

            # ---- Phase P2: V projection (natural layout) -> DRAM scratch
            with (
                tc.tile_pool(name="p2w", bufs=1) as p2w,
                tc.tile_pool(name="p2x", bufs=2) as p2x,
                tc.tile_pool(name="p2o", bufs=4) as p2o,
                tc.tile_pool(name="ps2", bufs=4, space="PSUM") as ps2,
            ):
                wv_s = p2w.tile([P, SK_T * GD], _f32r, tag="wv")
                for t in range(SK_T):
                    nc.sync.dma_start(wv_s[:, t * GD:(t + 1) * GD], wv[t * P:(t + 1) * P, :])
                for u in range(SQ_U):
                    xh = []
                    for half in range(2):
                        xu = p2x.tile([P, 8 * 512], _f32r, tag="xu")
                        for j in range(8):
                            t = half * 8 + j
                            nc.sync.dma_start(
                                xu[:, j * 512:(j + 1) * 512],
                                xT[t * P:(t + 1) * P, u * 512:(u + 1) * 512])
                        xh.append(xu)
                    for st in range(4):
                        g = 4 * u + st
                        pv = ps2.tile([P, GD], _f32, tag="pv")
                        for t in range(SK_T):
                            xu = xh[t // 8]
                            j = t % 8
                            nc.tensor.matmul(
                                pv[:],
                                xu[:, j * 512 + st * P: j * 512 + st * P + P],
                                wv_s[:, t * GD:(t + 1) * GD],
                                start=(t == 0), stop=(t == SK_T - 1))
                        vo = p2o.tile([P, GD], _f32r, tag="vo")
                        nc.scalar.copy(vo[:], pv[:])
                        nc.sync.dma_start(vd[:, g * GD:(g + 1) * GD], vo[:])

            # ---- Phases A (attention) + W (output projection)
            with (
                tc.tile_pool(name="amask", bufs=1 if causal else 2) as amask_p,
                tc.tile_pool(name="akv", bufs=2) as akv,
                tc.tile_pool(name="aq", bufs=2) as aq,
                tc.tile_pool(name="app", bufs=4) as app,
                tc.tile_pool(name="ar", bufs=2) as ar,
                tc.tile_pool(name="aw", bufs=1) as aw,
                tc.tile_pool(name="wst", bufs=3) as wst,
                tc.tile_pool(name="ps3", bufs=2, space="PSUM") as ps3,
                tc.tile_pool(name="ps4", bufs=2, space="PSUM") as ps4,
            ):
                wo_s = aw.tile([P, GH * S], _f32r, tag="wo")
                for dt in range(GH):
                    nc.sync.dma_start(wo_s[:, dt * S:(dt + 1) * S], wo[dt * P:(dt + 1) * P, :])
                if causal:
                    md_s = amask_p.tile([P, SK_T * 512], _f32, tag="md")
                    nc.sync.dma_start(md_s[:], maskd[:])
                for h in range(GH):
                    kt_h = akv.tile([P, S], _f32r, tag="kt")
                    nc.sync.dma_start(kt_h[:], ktd[:, h * S:(h + 1) * S])
                    v_h = akv.tile([P, S], _f32r, tag="vh")
                    for t in range(SK_T):
                        nc.sync.dma_start(
                            v_h[:, t * P:(t + 1) * P],
                            vd[:, t * GD + h * P: t * GD + (h + 1) * P])
                    for u in range(SQ_U):
                        qu = aq.tile([P, 512], _f32r, tag="qu")
                        nc.sync.dma_start(
                            qu[:], qtd[:, h * S + u * 512: h * S + (u + 1) * 512])
                        if not causal:
                            mu = amask_p.tile([P, SK_T * 512], _f32, tag="mu")
                            for t in range(SK_T):
                                nc.sync.dma_start(
                                    mu[:, t * 512:(t + 1) * 512],
                                    maskf[t * P:(t + 1) * P, u * 512:(u + 1) * 512])
                        n_sk = 4 * (u + 1) if causal else SK_T
                        psd = ps3.tile([P, 512], _f32, tag="d")
                        psa = ps3.tile([P, 512], _f32, tag="a")
                        pts = [None] * n_sk

                        def _consume(t):
                            nc.tensor.matmul(psd[:], ones_s[:], pts[t][:],
                                             start=(t == 0), stop=(t == n_sk - 1))
                            nc.tensor.matmul(psa[:], v_h[:, t * P:(t + 1) * P], pts[t][:],
                                             start=(t == 0), stop=(t == n_sk - 1))

                        for t in range(n_sk):
                            pss = ps3.tile([P, 512], _f32, tag="s")
                            nc.tensor.matmul(pss[:], kt_h[:, t * P:(t + 1) * P], qu[:],
                                             start=True, stop=True)
                            if causal:
                                if t >= 4 * u:  # diagonal-crossing blocks
                                    nc.vector.tensor_add(
                                        pss[:], pss[:], md_s[:, t * 512:(t + 1) * 512])
                            else:
                                nc.vector.tensor_add(
                                    pss[:], pss[:], mu[:, t * 512:(t + 1) * 512])
                            pt = app.tile([P, 512], _f32r, tag="p")
                            nc.scalar.activation(pt[:], pss[:],
                                                 mybir.ActivationFunctionType.Exp)
                            pts[t] = pt
                            if t > 0:
                                _consume(t - 1)  # software-pipeline: hide exp latency
                        _consume(n_sk - 1)
                        rec = ar.tile([P, 512], _f32, tag="rec")
                        nc.vector.reciprocal(rec[:], psd[:])
                        nc.vector.tensor_mul(
                            aot[:, h * S + u * 512: h * S + (u + 1) * 512],
                            psa[:], rec[:])
                # ---- W: out^T = wo^T.T @ AO^T
                for u in range(SQ_U):
                    for ot in range(SK_T):
                        po = ps4.tile([P, 512], _f32, tag="o")
                        for dt in range(GH):
                            nc.tensor.matmul(
                                po[:],
                                wo_s[:, dt * S + ot * P: dt * S + (ot + 1) * P],
                                aot[:, dt * S + u * 512: dt * S + (u + 1) * 512],
                                start=(dt == 0), stop=(dt == GH - 1))
                        so = wst.tile([P, 512], _f32, tag="so")
                        nc.scalar.copy(so[:], po[:])
                        nc.sync.dma_start(
                            outT[ot * P:(ot + 1) * P, u * 512:(u + 1) * 512], so[:])
    nc.compile()
    return nc


class _Runner:
    """Persistent PJRT executable for one compiled Bass module (SPMD over 8 cores)."""

    def __init__(self, nc, n_cores):
        import jax
        from jax.sharding import Mesh, PartitionSpec
        from jax.experimental.shard_map import shard_map
        from concourse.bass2jax import (
            _bass_exec_p, install_neuronx_cc_hook, partition_id_tensor)

        install_neuronx_cc_hook()
        self.jax = jax
        self.n_cores = n_cores
        partition_name = nc.partition_id_tensor.name if nc.partition_id_tensor else None
        in_names, out_names, out_avals = [], [], []
        for alloc in nc.m.functions[0].allocations:
            if not isinstance(alloc, mybir.MemoryLocationSet):
                continue
            name = alloc.memorylocations[0].name
            if alloc.kind == "ExternalInput":
                if name != partition_name:
                    in_names.append(name)
            elif alloc.kind == "ExternalOutput":
                out_names.append(name)
                out_avals.append(jax.core.ShapedArray(
                    tuple(alloc.tensor_shape), mybir.dt.np(alloc.dtype)))
        self.in_names, self.out_names, self.out_avals = in_names, out_names, out_avals
        n_params, n_outs = len(in_names), len(out_avals)
        all_in = list(in_names) + list(out_names)
        if partition_name is not None:
            all_in.append(partition_name)

        def _body(*args):
            operands = list(args)
            if partition_name is not None:
                operands.append(partition_id_tensor())
            return tuple(_bass_exec_p.bind(
                *operands,
                out_avals=tuple(out_avals), in_names=tuple(all_in),
                out_names=tuple(out_names), lowering_input_output_aliases=(),
                sim_require_finite=True, sim_require_nnan=True, nc=nc))

        devices = jax.devices()[:n_cores]
        mesh = Mesh(np.asarray(devices), ("core",))
        self.sharding = jax.sharding.NamedSharding(mesh, PartitionSpec("core"))
        self.fn = jax.jit(
            shard_map(_body, mesh=mesh,
                      in_specs=(PartitionSpec("core"),) * (n_params + n_outs),
                      out_specs=(PartitionSpec("core"),) * n_outs,
                      check_rep=False),
            keep_unused=True)
        self._dev_args = None

    def put_inputs(self, in_maps):
        jax = self.jax
        concat_in = [
            np.concatenate([np.asarray(in_maps[c][n]) for c in range(self.n_cores)], axis=0)
            for n in self.in_names]
        concat_zeros = [
            np.zeros((self.n_cores * a.shape[0], *a.shape[1:]), a.dtype)
            for a in self.out_avals]
        self._dev_args = [
            jax.device_put(v, self.sharding) for v in concat_in + concat_zeros]
        for a in self._dev_args:
            a.block_until_ready()

    def execute(self):
        return self.fn(*self._dev_args)

    def run(self, in_maps):
        self.put_inputs(in_maps)
        outs = self.execute()
        self.jax.block_until_ready(outs)
        return [
            {n: np.asarray(outs[i]).reshape(self.n_cores, *self.out_avals[i].shape)[c]
             for i, n in enumerate(self.out_names)}
            for c in range(self.n_cores)]


def _get_runner(causal: bool):
    if causal not in _cache:
        _cache[causal] = _Runner(_build(causal), NCORES)
    return _cache[causal]


def _host_prep(x, mask, Wq, Wk, Wv, Wo, causal):
    scale = np.float32(1.0) / np.sqrt(np.float32(HD))
    # de-interleave permutation within each head (cancels in Q.K^T)
    perm = np.concatenate(
        [np.concatenate([np.arange(0, HD, 2), np.arange(1, HD, 2)]) + HD * hh
         for hh in range(GH)])
    # RoPE tables, duplicated across halves; sin signed for fused add
    inv = (np.float32(1.0) / np.power(
        np.float32(10000.0),
        np.arange(0, HD, 2).astype(np.float32) / np.float32(HD))).astype(np.float32)
    ang = np.arange(S, dtype=np.float32)[:, None] * inv[None, :]   # [S, 64]
    cos_t = np.cos(ang).T.astype(np.float32)                       # [64, S]
    sin_t = np.sin(ang).T.astype(np.float32)
    cs_host = np.ascontiguousarray(np.concatenate([cos_t, cos_t], axis=0))
    ss_host = np.ascontiguousarray(np.concatenate([-sin_t, sin_t], axis=0))
    ones_host = np.ones((P, P), np.float32)
    maskT = np.ascontiguousarray(mask.T)
    if causal:
        md = np.empty((P, SK_T * 512), np.float32)
        for t in range(SK_T):
            u = t // 4
            md[:, t * 512:(t + 1) * 512] = maskT[t * P:(t + 1) * P, u * 512:(u + 1) * 512]
    xTs = [np.ascontiguousarray(x[b].T) for b in range(B)]
    in_maps = []
    for c in range(NCORES):
        b, g = c // G, c % G
        rows = slice(g * GD, (g + 1) * GD)
        m = {
            "xT": xTs[b],
            "wq": np.ascontiguousarray(Wq[rows].T[:, perm] * scale),
            "wk": np.ascontiguousarray(Wk[rows].T[:, perm]),
            "wv": np.ascontiguousarray(Wv[rows].T),
            "wo": np.ascontiguousarray(Wo[:, rows].T),
            "cs": cs_host,
            "ss": ss_host,
            "ones": ones_host,
        }
        if causal:
            m["maskd"] = md
        else:
            m["maskf"] = maskT
        in_maps.append(m)
    return in_maps


def kernel(x, mask, Wq, Wk, Wv, Wo):
    x = np.asarray(x, dtype=np.float32)
    mask = np.asarray(mask, dtype=np.float32)
    Wq = np.asarray(Wq, dtype=np.float32)
    Wk = np.asarray(Wk, dtype=np.float32)
    Wv = np.asarray(Wv, dtype=np.float32)
    Wo = np.asarray(Wo, dtype=np.float32)
    expected_mask = np.triu(np.full((S, S), -1e9, dtype=np.float32), k=1)
    causal = bool(np.array_equal(mask, expected_mask))
    runner = _get_runner(causal)
    in_maps = _host_prep(x, mask, Wq, Wk, Wv, Wo, causal)
    results = runner.run(in_maps)
    out = np.empty((B, S, D), np.float32)
    for b in range(B):
        acc = results[b * G]["outT"].copy()
        for g in range(1, G):
            acc += results[b * G + g]["outT"]
        out[b] = acc.T
    return out
